# revision 24
# baseline (speedup 1.0000x reference)
"""MoE (top-2 of 8 experts) Trainium2 kernel, expert-parallel across 8 NeuronCores.

Host side: gating matmul + top-2 + softmax + token dispatch (part of input
sharding). Device side (per core, SPMD): one expert's FFN over its routed
tokens in transposed layout —
    hT = gelu(w1e.T @ xeT + b1)        [H, C]
    yT = (w2e.T @ hT + b2) * wc        [D, C]
Host combines: out[token] += yT.T rows, summed over the 2 experts per token.

Matmuls run in bf16 (weights/activations), accumulate in fp32 PSUM.
"""

import numpy as np
import ml_dtypes

import jax
from jax.experimental.shard_map import shard_map
from jax.sharding import Mesh, PartitionSpec

import concourse.bass as bass  # noqa: F401
import concourse.mybir as mybir
import concourse.tile as tile
from concourse import bacc
from concourse.bass2jax import (
    _bass_exec_p,
    install_neuronx_cc_hook,
    partition_id_tensor,
)

B, S, D, H, E, TOPK = 4, 2048, 1024, 4096, 8, 2
T = B * S
P = 128
TB = 512  # token block (matmul free dim)

_BF16 = mybir.dt.bfloat16
_F32 = mybir.dt.float32

# cache of built+compiled runners keyed by capacity C, so repeat calls with
# the same shapes reuse the in-process jit executable (no re-trace/re-compile).
_RUNNER_CACHE: dict[int, tuple] = {}


def _make_runner(nc, n_cores=8):
    """Persistent jitted SPMD runner for a Bass module (mirrors
    concourse.bass2jax.run_bass_via_pjrt, but reusable across calls)."""
    install_neuronx_cc_hook()
    partition_name = nc.partition_id_tensor.name if nc.partition_id_tensor else None
    in_names, out_names, out_avals, zero_outs = [], [], [], []
    for alloc in nc.m.functions[0].allocations:
        if not isinstance(alloc, mybir.MemoryLocationSet):
            continue
        name = alloc.memorylocations[0].name
        if alloc.kind == "ExternalInput":
            if name != partition_name:
                in_names.append(name)
        elif alloc.kind == "ExternalOutput":
            out_names.append(name)
            shape = tuple(alloc.tensor_shape)
            dtype = mybir.dt.np(alloc.dtype)
            out_avals.append(jax.core.ShapedArray(shape, dtype))
            zero_outs.append(np.zeros(shape, dtype))
    n_params = len(in_names)
    all_in_names = list(in_names) + list(out_names)
    if partition_name is not None:
        all_in_names.append(partition_name)

    def _body(*args):
        operands = list(args)
        if partition_name is not None:
            operands.append(partition_id_tensor())
        outs = _bass_exec_p.bind(
            *operands,
            out_avals=tuple(out_avals),
            in_names=tuple(all_in_names),
            out_names=tuple(out_names),
            lowering_input_output_aliases=(),
            sim_require_finite=True,
            sim_require_nnan=True,
            nc=nc,
        )
        return tuple(outs)

    devices = jax.devices()[:n_cores]
    mesh = Mesh(np.asarray(devices), ("core",))
    n_outs = len(out_avals)
    in_specs = (PartitionSpec("core"),) * (n_params + n_outs)
    out_specs = (PartitionSpec("core"),) * n_outs
    f = jax.jit(
        shard_map(
            _body, mesh=mesh, in_specs=in_specs, out_specs=out_specs, check_rep=False
        ),
        donate_argnums=tuple(range(n_params, n_params + n_outs)),
        keep_unused=True,
    )

    def run(in_maps):
        concat_in = [
            np.concatenate([np.asarray(m[name]) for m in in_maps], axis=0)
            for name in in_names
        ]
        concat_zeros = [
            np.zeros((n_cores * z.shape[0], *z.shape[1:]), z.dtype) for z in zero_outs
        ]
        outs = f(*concat_in, *concat_zeros)
        return [
            {
                name: np.asarray(outs[i]).reshape(n_cores, *out_avals[i].shape)[c]
                for i, name in enumerate(out_names)
            }
            for c in range(n_cores)
        ]

    return run


def _build(C: int, reps: int = 1, w_chunk: int = 4096, x_bufs: int = 2,
           w1_split: int = 4096, w2_split: int = 1024, b2_zero: bool = False):
    """Bass module for one expert over C (padded) tokens.

    reps>1 repeats the FFN body (for wall-clock-differencing timing only).
    w_chunk: free-dim chunk size for weight DMAs (smaller chunks let the first
    matmuls start before the whole weight tile has landed).
    """
    nc = bacc.Bacc("TRN2", target_bir_lowering=False, debug=False, num_devices=8)

    xT = nc.dram_tensor("xT", [D, C], _BF16, kind="ExternalInput")
    w1 = nc.dram_tensor("w1", [D, H], _BF16, kind="ExternalInput")
    w2 = nc.dram_tensor("w2", [H, D], _BF16, kind="ExternalInput")
    b1t = nc.dram_tensor("b1t", [P, H // P], _F32, kind="ExternalInput")
    b2t = nc.dram_tensor("b2t", [P, D // P], _F32, kind="ExternalInput")
    wc = nc.dram_tensor("wc", [P, C], _F32, kind="ExternalInput")
    yT = nc.dram_tensor("yT", [D, C], _F32, kind="ExternalOutput")

    DK = D // P  # 8 k-tiles for mm1 (contraction over D)
    HK = H // P  # 32 k-tiles for mm2 (contraction over H)
    # split C evenly across blocks: a thin tail block would have matmul
    # free dims below the ~LDWEIGHTS-hiding threshold (weight loads become
    # exposed); even blocks keep every N large at identical total cost.
    n_blocks = (C + TB - 1) // TB
    base, rem = divmod(C, n_blocks)
    block_sizes = [base + (1 if i < rem else 0) for i in range(n_blocks)]
    block_starts = [sum(block_sizes[:i]) for i in range(n_blocks)]

    with tile.TileContext(nc) as tc:
        with (
            tc.tile_pool(name="wpool", bufs=1) as wpool,
            tc.tile_pool(name="xpool", bufs=x_bufs) as xpool,
            tc.tile_pool(name="hpool", bufs=1) as hpool,
            tc.tile_pool(name="ypool", bufs=3) as ypool,
            tc.tile_pool(name="psum1", bufs=4, space="PSUM") as psum1,
            tc.tile_pool(name="psum2", bufs=4, space="PSUM") as psum2,
        ):
            # DMA emission order is tuned for startup: block-0 activations
            # first (small, everything depends on them), then w1 chunks in
            # the order mm1 consumes them, then w2 (not needed until mm2,
            # ~100us in), then the small epilogue constants. Weights are
            # resident in SBUF as separate chunk tiles; a chunk's consumers
            # only depend on that chunk's DMA, so early matmuls start before
            # the whole weight matrix lands.
            x_tiles = {}

            def load_x(ib):
                t0 = block_starts[ib]
                tw = block_sizes[ib]
                xt = xpool.tile([P, DK, TB], _BF16, name="x_sb")
                for dk in range(DK):
                    nc.sync.dma_start(
                        xt[:, dk, :tw], xT[dk * P : (dk + 1) * P, t0 : t0 + tw]
                    )
                x_tiles[ib] = xt

            load_x(0)

            w1_sb = {}  # (dk, j) -> tile covering w1[dk*P:(dk+1)*P, j*w1_split:...]
            for j in range(0, H, w1_split):
                for dk in range(DK):
                    t = wpool.tile(
                        [P, min(w1_split, H - j)], _BF16, name=f"w1_sb{dk}_{j}"
                    )
                    nc.sync.dma_start(
                        t[:], w1[dk * P : (dk + 1) * P, j : j + t.shape[1]]
                    )
                    w1_sb[(dk, j // w1_split)] = t

            w2_sb = {}
            for j in range(0, D, w2_split):
                for hk in range(HK):
                    t = wpool.tile(
                        [P, min(w2_split, D - j)], _BF16, name=f"w2_sb{hk}_{j}"
                    )
                    nc.sync.dma_start(
                        t[:], w2[hk * P : (hk + 1) * P, j : j + t.shape[1]]
                    )
                    w2_sb[(hk, j // w2_split)] = t

            b1_sb = wpool.tile([P, H // P], _F32, name="b1_sb")
            nc.sync.dma_start(b1_sb[:], b1t[:])
            b2_sb = wpool.tile([P, D // P], _F32, name="b2_sb")
            nc.sync.dma_start(b2_sb[:], b2t[:])
            wc_sb = wpool.tile([P, C], _F32, name="wc_sb")
            nc.sync.dma_start(wc_sb[:], wc[:])

            for _rep in range(reps):
              for ib in range(n_blocks):
                t0 = block_starts[ib]
                tw = block_sizes[ib]

                if ib not in x_tiles:
                    load_x(ib)
                x_sb = x_tiles.pop(ib)
                if ib + 1 < n_blocks:
                    load_x(ib + 1)  # prefetch next block's tokens

                h_sb = hpool.tile([P, HK, TB], _BF16, name="h_sb")

                # mm1: hT[hm-tile, tokens] = sum_dk w1[dk, hm].T @ xT[dk, tokens]
                for hm in range(HK):
                    ps = psum1.tile([P, TB], _F32, name="ps1")
                    j, off = divmod(hm * P, w1_split)
                    for dk in range(DK):
                        nc.tensor.matmul(
                            ps[:, :tw],
                            w1_sb[(dk, j)][:, off : off + P],
                            x_sb[:, dk, :tw],
                            start=(dk == 0),
                            stop=(dk == DK - 1),
                        )
                    nc.scalar.activation(
                        h_sb[:, hm, :tw],
                        ps[:, :tw],
                        mybir.ActivationFunctionType.Gelu,
                        bias=b1_sb[:, hm : hm + 1],
                    )

                # mm2: yT[dm-tile, tokens] = sum_hk w2[hk, dm].T @ hT[hk, tokens]
                for dm in range(DK):
                    ps2 = psum2.tile([P, TB], _F32, name="ps2")
                    j, off = divmod(dm * P, w2_split)
                    for hk in range(HK):
                        nc.tensor.matmul(
                            ps2[:, :tw],
                            w2_sb[(hk, j)][:, off : off + P],
                            h_sb[:, hk, :tw],
                            start=(hk == 0),
                            stop=(hk == HK - 1),
                        )
                    y_sb = ypool.tile([P, TB], _F32, name="y_sb")
                    if b2_zero:
                        # (ps2 + 0) * wc in one DVE op straight from PSUM
                        nc.vector.tensor_mul(
                            out=y_sb[:, :tw],
                            in0=ps2[:, :tw],
                            in1=wc_sb[:, t0 : t0 + tw],
                        )
                    else:
                        nc.scalar.activation(
                            y_sb[:, :tw],
                            ps2[:, :tw],
                            mybir.ActivationFunctionType.Identity,
                            bias=b2_sb[:, dm : dm + 1],
                        )
                        nc.vector.tensor_mul(
                            out=y_sb[:, :tw],
                            in0=y_sb[:, :tw],
                            in1=wc_sb[:, t0 : t0 + tw],
                        )
                    nc.sync.dma_start(
                        yT[dm * P : (dm + 1) * P, t0 : t0 + tw], y_sb[:, :tw]
                    )

    nc.compile()
    return nc


def _route(xf, gate_w, gate_b):
    """Top-2 gating in numpy. Returns per-expert (token_ids, combine_weights)."""
    gates = xf @ gate_w + gate_b  # [T, E] f32
    i1 = np.argmax(gates, axis=1)
    v1 = gates[np.arange(T), i1]
    masked = gates.copy()
    masked[np.arange(T), i1] = -np.inf
    i2 = np.argmax(masked, axis=1)
    v2 = masked[np.arange(T), i2]
    # softmax over the two top scores (v1 >= v2)
    e2 = np.exp(v2 - v1)
    g1 = 1.0 / (1.0 + e2)
    g2 = e2 / (1.0 + e2)

    tok_ids, tok_w = [], []
    for e in range(E):
        m1 = i1 == e
        m2 = i2 == e
        ids = np.concatenate([np.nonzero(m1)[0], np.nonzero(m2)[0]])
        w = np.concatenate([g1[m1], g2[m2]]).astype(np.float32)
        tok_ids.append(ids)
        tok_w.append(w)
    return tok_ids, tok_w


def kernel(x, gate_w, gate_b, w1, b1, w2, b2):
    x = np.asarray(x, dtype=np.float32)
    gate_w = np.asarray(gate_w, dtype=np.float32)
    gate_b = np.asarray(gate_b, dtype=np.float32)
    w1 = np.asarray(w1, dtype=np.float32)
    b1 = np.asarray(b1, dtype=np.float32)
    w2 = np.asarray(w2, dtype=np.float32)
    b2 = np.asarray(b2, dtype=np.float32)

    xf = x.reshape(T, D)
    tok_ids, tok_w = _route(xf, gate_w, gate_b)

    max_count = max(len(ids) for ids in tok_ids)
    C = max(TB, max_count)
    b2_zero = not b2.any()

    key = (C, b2_zero)
    if key not in _RUNNER_CACHE:
        _RUNNER_CACHE[key] = _make_runner(
            _build(C, w1_split=1024, b2_zero=b2_zero)
        )
    run = _RUNNER_CACHE[key]

    in_maps = []
    for e in range(E):
        ids = tok_ids[e]
        cnt = len(ids)
        xe = np.zeros((C, D), dtype=np.float32)
        xe[:cnt] = xf[ids]
        wce = np.zeros((C,), dtype=np.float32)
        wce[:cnt] = tok_w[e]
        in_maps.append(
            {
                "xT": np.ascontiguousarray(xe.T).astype(ml_dtypes.bfloat16),
                "w1": w1[e].astype(ml_dtypes.bfloat16),
                "w2": w2[e].astype(ml_dtypes.bfloat16),
                "b1t": np.ascontiguousarray(b1[e].reshape(H // P, P).T),
                "b2t": np.ascontiguousarray(b2[e].reshape(D // P, P).T),
                "wc": np.broadcast_to(wce, (P, C)).copy(),
            }
        )

    results = run(in_maps)

    out = np.zeros((T, D), dtype=np.float32)
    for e in range(E):
        ids = tok_ids[e]
        cnt = len(ids)
        ye = results[e]["yT"][:, :cnt].T  # [cnt, D]
        out[ids] += ye
    return out.reshape(B, S, D)
